# revision 15
# baseline (speedup 1.0000x reference)
"""Trainium2 Bass kernel for nn_Decoder (LSTM over T steps + final FC).

Problem: y_hist [256, 512], LSTM(input_size=1, hidden=1024), h0/c0 [256, 1024],
output = h_T @ W_fc.T + b_fc -> [256, 1].

Sharding: data-parallel. Batch 256 -> 8 cores x 32 rows. LSTM + fc weights
replicated on every core; the time recurrence stays local per core (no
collectives).

Per-core per-step compute (batch-in-partitions orientation, column-tiled):
  gates[32, 4096] = h[32,1024] @ W_hh^T  (+ x_t * w_in + bias)
  - The 128x128 PE array runs 4 concurrent M=32 matmuls via column tiling
    (tile_position=(0, 32q)); operands bf16 (walrus rejects col-tiled
    fp32/f32r), accumulation fp32 in PSUM. Cell state c stays fp32.
  - PE column group q (PSUM partitions 32q:32q+32) computes all four gates
    for H-quarter q. PSUM PS [128, 1024]: cols [0:256)=i, [256:512)=f,
    [512:768)=g, [768:1024)=o; partition 32q+b <-> (batch b, H-col 256q+n).
    So i/f/g/o/c are all partition-aligned [128, 256] tiles (per-lane
    engines cannot cross partitions).
  - x_t*w_in + bias enters as a K=2 matmul (rows {x_t, ones} x {w_in, bias})
    with start=True; the 8 K-tiles of h @ W_hh^T accumulate on top.
  - h_new [128, 256] is re-transposed to hT (h^T, K-tile-major with column
    order HT_ORDER) by 4 PE transposes of [64,128] blocks for the next
    step's stationary operand.
  - Final FC: per-partition dot + reduce; the cross-partition 4-way add is
    done exactly via a DRAM round-trip rearrange [128,1] -> [32,4].
"""

import numpy as np
import ml_dtypes

import concourse.bass as bass
import concourse.mybir as mybir
from concourse import bacc
from concourse.tile import TileContext
from concourse.bass_utils import run_bass_kernel_spmd

B, T, H = 256, 512, 1024
NCORES = 8
BL = B // NCORES  # 32 batch rows per core
KT = H // 128  # 8 contraction tiles
F32 = mybir.dt.float32
BF16 = mybir.dt.bfloat16
NPBF16 = ml_dtypes.bfloat16

X1_CHUNK = 64  # steps per x1 SBUF prefetch chunk

# hT column layout: K-tiles stored in order [0,2,4,6,1,3,5,7] (32 cols each).
# Full [128,128] transpose of h_new cols [128c:128c+128] yields tiles
# {2q+c for q in 0..3} as its four 32-col groups.
HT_ORDER = [0, 2, 4, 6, 1, 3, 5, 7]
HT_COL = {k: 32 * HT_ORDER.index(k) for k in range(8)}

# bf16 packed blob column offsets
PKB_WT = 0
PKB_XB = PKB_WT + KT * 4096
PKB_HT0 = PKB_XB + 4096
PKB_IDN = PKB_HT0 + KT * BL
PKB_COLS = PKB_IDN + 128

# f32 packed blob column offsets
PKF_C0 = 0
PKF_WFC = PKF_C0 + 256
PKF_BFC = PKF_WFC + 256
PKF_COLS = PKF_BFC + 1


def build_nc(n_steps: int = T, repeat: int = 1) -> bass.Bass:
    nc = bacc.Bacc()

    initb_d = nc.declare_dram_parameter("initb", [128, PKB_COLS], BF16, isOutput=False)
    initf_d = nc.declare_dram_parameter("initf", [128, PKF_COLS], F32, isOutput=False)
    x1_d = nc.declare_dram_parameter("x1", [2, n_steps * BL], BF16, isOutput=False)
    out_d = nc.declare_dram_parameter("out", [BL, 1], F32, isOutput=True)
    scr_d = nc.dram_tensor("scratch", [128], F32)

    with TileContext(nc) as tc:
        with (
            tc.tile_pool(name="consts", bufs=1) as consts,
            tc.tile_pool(name="state", bufs=1) as state,
            tc.tile_pool(name="x1pool", bufs=2) as x1pool,
            tc.tile_pool(name="work", bufs=2) as work,
            tc.tile_pool(name="psum", bufs=2, space="PSUM") as psum,
            tc.tile_pool(name="psumt", bufs=2, space="PSUM") as psumt,
        ):
            pkb = consts.tile([128, PKB_COLS], BF16)
            nc.sync.dma_start(out=pkb, in_=initb_d[:, :])
            pkf = consts.tile([128, PKF_COLS], F32)
            nc.sync.dma_start(out=pkf, in_=initf_d[:, :])
            wt_sb = pkb[:, PKB_WT : PKB_WT + KT * 4096]
            xb_sb = pkb[0:2, PKB_XB : PKB_XB + 4096]
            idn128 = pkb[:, PKB_IDN : PKB_IDN + 128]
            wfc_sb = pkf[:, PKF_WFC : PKF_WFC + 256]
            bfc_sb = pkf[0:BL, PKF_BFC : PKF_BFC + 1]

            # Repeat loop (timing harness: re-runs the whole recurrence).
            rep_ctx = tc.For_i(0, repeat, 1) if repeat > 1 else None
            if rep_ctx is not None:
                rep_ctx.__enter__()

            # Mutable state: copied out of the packed blobs on-chip.
            hT = state.tile([128, KT * BL], BF16)
            nc.vector.tensor_copy(hT, pkb[:, PKB_HT0 : PKB_HT0 + KT * BL])
            c_sb = state.tile([128, 256], F32)
            nc.vector.tensor_copy(c_sb, pkf[:, PKF_C0 : PKF_C0 + 256])

            x1c = None
            hnew = None
            for t in range(n_steps):
                u = t % X1_CHUNK
                if u == 0:
                    nst = min(X1_CHUNK, n_steps - t)
                    x1c = x1pool.tile([2, X1_CHUNK * BL], BF16, name="x1c")
                    nc.sync.dma_start(
                        out=x1c[:, : nst * BL],
                        in_=x1_d[:, t * BL : (t + nst) * BL],
                    )
                xsl = x1c[:, u * BL : (u + 1) * BL]

                psA = psum.tile([128, 512], F32, name="psA")
                psB = psum.tile([128, 512], F32, name="psB")
                pshalf = (psA, psB)

                # K=2 matmul: gates = x_t * w_in + bias  (start of group)
                for half in range(2):
                    for q in range(4):
                        nc.tensor.matmul(
                            pshalf[half][32 * q : 32 * q + 32, :],
                            xsl,
                            xb_sb[
                                :, 1024 * q + 512 * half : 1024 * q + 512 * half + 512
                            ],
                            start=True,
                            stop=False,
                            tile_position=(0, 32 * q),
                            skip_group_check=True,
                        )

                # Main recurrence matmul: 8 K-tiles x (4 col groups x 2 halves)
                for k in range(KT):
                    lt = hT[:, HT_COL[k] : HT_COL[k] + BL]
                    for half in range(2):
                        for q in range(4):
                            base = 4096 * k + 1024 * q + 512 * half
                            nc.tensor.matmul(
                                pshalf[half][32 * q : 32 * q + 32, :],
                                lt,
                                wt_sb[:, base : base + 512],
                                start=False,
                                stop=(k == KT - 1),
                                tile_position=(0, 32 * q),
                                skip_group_check=True,
                            )

                # Elementwise: cols [0:256)=i [256:512)=f [512:768)=g [768:1024)=o
                sif = work.tile([128, 512], F32, name="sif")
                nc.scalar.activation(
                    sif, psA, mybir.ActivationFunctionType.Sigmoid
                )
                tg = work.tile([128, 256], F32, name="tg")
                nc.scalar.activation(
                    tg, psB[:, 0:256], mybir.ActivationFunctionType.Tanh
                )
                so = work.tile([128, 256], F32, name="so")
                nc.scalar.activation(
                    so, psB[:, 256:512], mybir.ActivationFunctionType.Sigmoid
                )
                t1 = work.tile([128, 256], F32, name="t1")
                nc.vector.tensor_mul(t1, sif[:, 256:512], c_sb)
                t2 = work.tile([128, 256], F32, name="t2")
                nc.vector.tensor_mul(t2, sif[:, 0:256], tg)
                nc.vector.tensor_add(c_sb, t1, t2)
                th = work.tile([128, 256], F32, name="th")
                nc.scalar.activation(th, c_sb, mybir.ActivationFunctionType.Tanh)
                hnew = work.tile([128, 256], BF16, name="hnew")
                nc.vector.tensor_mul(hnew, so, th)

                # Transpose h_new -> hT for next step: 2 full [128,128]
                # PE transposes (base partition 0 only; mixing LDW base
                # partitions between transposes wedges the device).
                if t != n_steps - 1:
                    psT = psumt.tile([128, KT * BL], BF16, name="psT")
                    for c in range(2):
                        nc.tensor.matmul(
                            psT[:, 128 * c : 128 * c + 128],
                            hnew[:, 128 * c : 128 * c + 128],
                            idn128,
                            is_transpose=True,
                            start=True,
                            stop=True,
                            skip_group_check=True,
                        )
                    nc.vector.tensor_copy(hT, psT)

            # Final FC: out[b] = sum_H h[b,H]*wfc[H] + b_fc
            fcm = work.tile([128, 256], F32)
            nc.vector.tensor_mul(fcm, hnew, wfc_sb)
            fcrf = work.tile([128, 1], F32)
            nc.vector.reduce_sum(out=fcrf, in_=fcm, axis=mybir.AxisListType.X)
            # exact cross-partition 4-way add via DRAM round-trip rearrange
            nc.sync.dma_start(out=scr_d[:], in_=fcrf[:, 0])
            fcr4 = work.tile([BL, 4], F32)
            nc.sync.dma_start(
                out=fcr4, in_=scr_d.ap().rearrange("(q b) -> b q", b=BL)
            )
            fco = work.tile([BL, 1], F32)
            nc.vector.reduce_sum(out=fco, in_=fcr4, axis=mybir.AxisListType.X)
            outsb = work.tile([BL, 1], F32)
            nc.vector.tensor_scalar_add(outsb, fco, scalar1=bfc_sb)
            nc.sync.dma_start(out=out_d[:, :], in_=outsb)
            if rep_ctx is not None:
                rep_ctx.__exit__(None, None, None)

    nc.compile()
    return nc


def prep_inputs(y_hist, W_ih, W_hh, b_ih, b_hh, W_fc, b_fc, h0, c0, n_steps: int = T):
    """Build the 8 per-core input maps (host-side numpy re-layouts)."""
    f = np.float32
    W_hh = np.asarray(W_hh, f)
    w_in = np.asarray(W_ih, f)[:, 0]
    bias = (np.asarray(b_ih, f) + np.asarray(b_hh, f)).astype(f)
    W_fc = np.asarray(W_fc, f)
    b_fc = np.asarray(b_fc, f)
    y_hist = np.asarray(y_hist, f)
    h0 = np.asarray(h0, f)
    c0 = np.asarray(c0, f)

    # wt[p, 4096k + 1024q + 256gi + n] = W_hh[1024gi + 256q + n, 128k + p]
    wt = np.zeros((128, KT * 4096), f)
    xb = np.zeros((2, 4096), f)
    for q in range(4):
        for gi in range(4):
            src = slice(1024 * gi + 256 * q, 1024 * gi + 256 * q + 256)
            for k in range(KT):
                dst = slice(
                    4096 * k + 1024 * q + 256 * gi,
                    4096 * k + 1024 * q + 256 * gi + 256,
                )
                wt[:, dst] = W_hh[src, 128 * k : 128 * (k + 1)].T
            xb[0, 1024 * q + 256 * gi : 1024 * q + 256 * gi + 256] = w_in[src]
            xb[1, 1024 * q + 256 * gi : 1024 * q + 256 * gi + 256] = bias[src]

    wfc = np.vstack(
        [np.tile(W_fc[0, 256 * q : 256 * (q + 1)], (32, 1)) for q in range(4)]
    ).astype(f)
    bfc = float(np.asarray(b_fc).reshape(-1)[0])
    idn128 = np.eye(128, dtype=f)

    in_maps = []
    for i in range(NCORES):
        b0 = BL * i
        ys = y_hist[b0 : b0 + BL, :n_steps]  # [32, n_steps]
        x1 = np.stack([ys.T.reshape(-1), np.ones(n_steps * BL, f)])
        h0s = h0[b0 : b0 + BL]
        ht0 = np.concatenate(
            [h0s[:, 128 * k : 128 * (k + 1)].T for k in HT_ORDER], axis=1
        )
        c0s = c0[b0 : b0 + BL]
        c0l = np.vstack([c0s[:, 256 * q : 256 * (q + 1)] for q in range(4)])

        pkb = np.zeros((128, PKB_COLS), NPBF16)
        pkb[:, PKB_WT : PKB_WT + KT * 4096] = wt.astype(NPBF16)
        pkb[0:2, PKB_XB : PKB_XB + 4096] = xb.astype(NPBF16)
        pkb[:, PKB_HT0 : PKB_HT0 + KT * BL] = ht0.astype(NPBF16)
        pkb[:, PKB_IDN : PKB_IDN + 128] = idn128.astype(NPBF16)

        pkf = np.zeros((128, PKF_COLS), f)
        pkf[:, PKF_C0 : PKF_C0 + 256] = c0l
        pkf[:, PKF_WFC : PKF_WFC + 256] = wfc
        pkf[0:BL, PKF_BFC] = bfc

        in_maps.append(
            {
                "initb": np.ascontiguousarray(pkb),
                "initf": np.ascontiguousarray(pkf),
                "x1": np.ascontiguousarray(x1.astype(NPBF16)),
            }
        )
    return in_maps


def run(inputs: dict, n_steps: int = T, trace: bool = False):
    nc = build_nc(n_steps)
    in_maps = prep_inputs(**inputs, n_steps=n_steps)
    res = run_bass_kernel_spmd(nc, in_maps, list(range(NCORES)), trace=trace)
    out = np.concatenate([res.results[i]["out"] for i in range(NCORES)], axis=0)
    return out, res


def kernel(**inputs) -> np.ndarray:
    out, _ = run(inputs, n_steps=T)
    return out


# revision 16
# speedup vs baseline: 1.5641x; 1.5641x over previous
"""Trainium2 Bass kernel for nn_Decoder (LSTM over T steps + final FC).

Problem: y_hist [256, 512], LSTM(input_size=1, hidden=1024), h0/c0 [256, 1024],
output = h_T @ W_fc.T + b_fc -> [256, 1].

Sharding: data-parallel. Batch 256 -> 8 cores x 32 rows. LSTM + fc weights
replicated on every core; the time recurrence stays local per core (no
collectives).

Per-core per-step compute (batch-in-partitions orientation, column-tiled):
  gates[32, 4096] = h[32,1024] @ W_hh^T  (+ x_t * w_in + bias)
  - The 128x128 PE array runs 4 concurrent M=32 matmuls via column tiling
    (tile_position=(0, 32q)); operands bf16 (walrus rejects col-tiled
    fp32/f32r), accumulation fp32 in PSUM. Cell state c stays fp32.
  - PE column group q (PSUM partitions 32q:32q+32) computes all four gates
    for H-quarter q. PSUM PS [128, 1024]: cols [0:256)=i, [256:512)=f,
    [512:768)=g, [768:1024)=o; partition 32q+b <-> (batch b, H-col 256q+n).
    So i/f/g/o/c are all partition-aligned [128, 256] tiles (per-lane
    engines cannot cross partitions).
  - x_t*w_in + bias enters as a K=2 matmul (rows {x_t, ones} x {w_in, bias})
    with start=True; the 8 K-tiles of h @ W_hh^T accumulate on top.
  - h_new [128, 256] is re-transposed to hT (h^T, K-tile-major with column
    order HT_ORDER) by 4 PE transposes of [64,128] blocks for the next
    step's stationary operand.
  - Final FC: per-partition dot + reduce; the cross-partition 4-way add is
    done exactly via a DRAM round-trip rearrange [128,1] -> [32,4].
"""

import numpy as np
import ml_dtypes

import concourse.bass as bass
import concourse.mybir as mybir
from concourse import bacc
from concourse.tile import TileContext
from concourse.bass_utils import run_bass_kernel_spmd

B, T, H = 256, 512, 1024
NCORES = 8
BL = B // NCORES  # 32 batch rows per core
KT = H // 128  # 8 contraction tiles
F32 = mybir.dt.float32
BF16 = mybir.dt.bfloat16
NPBF16 = ml_dtypes.bfloat16

X1_CHUNK = 64  # steps per x1 SBUF prefetch chunk

# hT column layout: K-tiles stored in order [0,2,4,6,1,3,5,7] (32 cols each).
# Full [128,128] transpose of h_new cols [128c:128c+128] yields tiles
# {2q+c for q in 0..3} as its four 32-col groups.
HT_ORDER = [0, 2, 4, 6, 1, 3, 5, 7]
HT_COL = {k: 32 * HT_ORDER.index(k) for k in range(8)}

# bf16 packed blob column offsets
PKB_WT = 0
PKB_XB = PKB_WT + KT * 4096
PKB_HT0 = PKB_XB + 4096
PKB_IDN = PKB_HT0 + KT * BL
PKB_COLS = PKB_IDN + 128

# f32 packed blob column offsets
PKF_C0 = 0
PKF_WFC = PKF_C0 + 256
PKF_BFC = PKF_WFC + 256
PKF_COLS = PKF_BFC + 1


def build_nc(n_steps: int = T, repeat: int = 1, dbg_skip_elem: bool = False) -> bass.Bass:
    nc = bacc.Bacc()

    initb_d = nc.declare_dram_parameter("initb", [128, PKB_COLS], BF16, isOutput=False)
    initf_d = nc.declare_dram_parameter("initf", [128, PKF_COLS], F32, isOutput=False)
    x1_d = nc.declare_dram_parameter("x1", [2, n_steps * BL], BF16, isOutput=False)
    out_d = nc.declare_dram_parameter("out", [BL, 1], F32, isOutput=True)
    scr_d = nc.dram_tensor("scratch", [128], F32)

    with TileContext(nc) as tc:
        with (
            tc.tile_pool(name="consts", bufs=1) as consts,
            tc.tile_pool(name="state", bufs=1) as state,
            tc.tile_pool(name="x1pool", bufs=2) as x1pool,
            tc.tile_pool(name="work", bufs=2) as work,
            tc.tile_pool(name="psum", bufs=2, space="PSUM") as psum,
            tc.tile_pool(name="psumt", bufs=2, space="PSUM") as psumt,
        ):
            pkb = consts.tile([128, PKB_COLS], BF16)
            nc.sync.dma_start(out=pkb, in_=initb_d[:, :])
            pkf = consts.tile([128, PKF_COLS], F32)
            nc.sync.dma_start(out=pkf, in_=initf_d[:, :])
            wt_sb = pkb[:, PKB_WT : PKB_WT + KT * 4096]
            xb_sb = pkb[0:2, PKB_XB : PKB_XB + 4096]
            idn128 = pkb[:, PKB_IDN : PKB_IDN + 128]
            wfc_sb = pkf[:, PKF_WFC : PKF_WFC + 256]
            bfc_sb = pkf[0:BL, PKF_BFC : PKF_BFC + 1]

            # Repeat loop (timing harness: re-runs the whole recurrence).
            rep_ctx = tc.For_i(0, repeat, 1) if repeat > 1 else None
            if rep_ctx is not None:
                rep_ctx.__enter__()

            # Mutable state: copied out of the packed blobs on-chip.
            hT = state.tile([128, KT * BL], BF16)
            nc.vector.tensor_copy(hT, pkb[:, PKB_HT0 : PKB_HT0 + KT * BL])
            c_sb = state.tile([128, 256], F32)
            nc.vector.tensor_copy(c_sb, pkf[:, PKF_C0 : PKF_C0 + 256])

            x1c = None
            hnew = None
            for t in range(n_steps):
                u = t % X1_CHUNK
                if u == 0:
                    nst = min(X1_CHUNK, n_steps - t)
                    x1c = x1pool.tile([2, X1_CHUNK * BL], BF16, name="x1c")
                    nc.sync.dma_start(
                        out=x1c[:, : nst * BL],
                        in_=x1_d[:, t * BL : (t + nst) * BL],
                    )
                xsl = x1c[:, u * BL : (u + 1) * BL]

                psA = psum.tile([128, 512], F32, name="psA")
                psB = psum.tile([128, 512], F32, name="psB")
                pshalf = (psA, psB)

                # K=2 matmul: gates = x_t * w_in + bias  (start of group)
                for half in range(2):
                    for q in range(4):
                        nc.tensor.matmul(
                            pshalf[half][32 * q : 32 * q + 32, :],
                            xsl,
                            xb_sb[
                                :, 1024 * q + 512 * half : 1024 * q + 512 * half + 512
                            ],
                            start=True,
                            stop=False,
                            tile_position=(0, 32 * q),
                            skip_group_check=True,
                        )

                # Main recurrence matmul: 8 K-tiles x (4 col groups x 2 halves)
                for k in range(KT):
                    lt = hT[:, HT_COL[k] : HT_COL[k] + BL]
                    for half in range(2):
                        for q in range(4):
                            base = 4096 * k + 1024 * q + 512 * half
                            nc.tensor.matmul(
                                pshalf[half][32 * q : 32 * q + 32, :],
                                lt,
                                wt_sb[:, base : base + 512],
                                start=False,
                                stop=(k == KT - 1),
                                tile_position=(0, 32 * q),
                                skip_group_check=True,
                            )

                if dbg_skip_elem:
                    # timing probe: skip ACT/DVE chain, transpose a const
                    if t != n_steps - 1:
                        psT = psumt.tile([128, KT * BL], BF16, name="psT")
                        for c in range(2):
                            nc.tensor.matmul(
                                psT[:, 128 * c : 128 * c + 128],
                                pkb[:, 128 * c : 128 * c + 128],
                                idn128,
                                is_transpose=True,
                                start=True,
                                stop=True,
                                skip_group_check=True,
                            )
                        nc.vector.tensor_copy(hT, psT)
                    continue

                # Elementwise: cols [0:256)=i [256:512)=f [512:768)=g [768:1024)=o
                sif = work.tile([128, 512], F32, name="sif")
                nc.scalar.activation(
                    sif, psA, mybir.ActivationFunctionType.Sigmoid
                )
                tg = work.tile([128, 256], F32, name="tg")
                nc.scalar.activation(
                    tg, psB[:, 0:256], mybir.ActivationFunctionType.Tanh
                )
                so = work.tile([128, 256], F32, name="so")
                nc.scalar.activation(
                    so, psB[:, 256:512], mybir.ActivationFunctionType.Sigmoid
                )
                t1 = work.tile([128, 256], F32, name="t1")
                nc.vector.tensor_mul(t1, sif[:, 256:512], c_sb)
                t2 = work.tile([128, 256], F32, name="t2")
                nc.vector.tensor_mul(t2, sif[:, 0:256], tg)
                nc.vector.tensor_add(c_sb, t1, t2)
                th = work.tile([128, 256], F32, name="th")
                nc.scalar.activation(th, c_sb, mybir.ActivationFunctionType.Tanh)
                hnew = work.tile([128, 256], BF16, name="hnew")
                nc.vector.tensor_mul(hnew, so, th)

                # Transpose h_new -> hT for next step: 2 full [128,128]
                # PE transposes (base partition 0 only; mixing LDW base
                # partitions between transposes wedges the device).
                if t != n_steps - 1:
                    psT = psumt.tile([128, KT * BL], BF16, name="psT")
                    for c in range(2):
                        nc.tensor.matmul(
                            psT[:, 128 * c : 128 * c + 128],
                            hnew[:, 128 * c : 128 * c + 128],
                            idn128,
                            is_transpose=True,
                            start=True,
                            stop=True,
                            skip_group_check=True,
                        )
                    nc.vector.tensor_copy(hT, psT)

            # Final FC: out[b] = sum_H h[b,H]*wfc[H] + b_fc
            if hnew is None:
                hnew = c_sb
            fcm = work.tile([128, 256], F32)
            nc.vector.tensor_mul(fcm, hnew, wfc_sb)
            fcrf = work.tile([128, 1], F32)
            nc.vector.reduce_sum(out=fcrf, in_=fcm, axis=mybir.AxisListType.X)
            # exact cross-partition 4-way add via DRAM round-trip rearrange
            nc.sync.dma_start(out=scr_d[:], in_=fcrf[:, 0])
            fcr4 = work.tile([BL, 4], F32)
            nc.sync.dma_start(
                out=fcr4, in_=scr_d.ap().rearrange("(q b) -> b q", b=BL)
            )
            fco = work.tile([BL, 1], F32)
            nc.vector.reduce_sum(out=fco, in_=fcr4, axis=mybir.AxisListType.X)
            outsb = work.tile([BL, 1], F32)
            nc.vector.tensor_scalar_add(outsb, fco, scalar1=bfc_sb)
            nc.sync.dma_start(out=out_d[:, :], in_=outsb)
            if rep_ctx is not None:
                rep_ctx.__exit__(None, None, None)

    nc.compile()
    return nc


def prep_inputs(y_hist, W_ih, W_hh, b_ih, b_hh, W_fc, b_fc, h0, c0, n_steps: int = T):
    """Build the 8 per-core input maps (host-side numpy re-layouts)."""
    f = np.float32
    W_hh = np.asarray(W_hh, f)
    w_in = np.asarray(W_ih, f)[:, 0]
    bias = (np.asarray(b_ih, f) + np.asarray(b_hh, f)).astype(f)
    W_fc = np.asarray(W_fc, f)
    b_fc = np.asarray(b_fc, f)
    y_hist = np.asarray(y_hist, f)
    h0 = np.asarray(h0, f)
    c0 = np.asarray(c0, f)

    # wt[p, 4096k + 1024q + 256gi + n] = W_hh[1024gi + 256q + n, 128k + p]
    wt = np.zeros((128, KT * 4096), f)
    xb = np.zeros((2, 4096), f)
    for q in range(4):
        for gi in range(4):
            src = slice(1024 * gi + 256 * q, 1024 * gi + 256 * q + 256)
            for k in range(KT):
                dst = slice(
                    4096 * k + 1024 * q + 256 * gi,
                    4096 * k + 1024 * q + 256 * gi + 256,
                )
                wt[:, dst] = W_hh[src, 128 * k : 128 * (k + 1)].T
            xb[0, 1024 * q + 256 * gi : 1024 * q + 256 * gi + 256] = w_in[src]
            xb[1, 1024 * q + 256 * gi : 1024 * q + 256 * gi + 256] = bias[src]

    wfc = np.vstack(
        [np.tile(W_fc[0, 256 * q : 256 * (q + 1)], (32, 1)) for q in range(4)]
    ).astype(f)
    bfc = float(np.asarray(b_fc).reshape(-1)[0])
    idn128 = np.eye(128, dtype=f)

    in_maps = []
    for i in range(NCORES):
        b0 = BL * i
        ys = y_hist[b0 : b0 + BL, :n_steps]  # [32, n_steps]
        x1 = np.stack([ys.T.reshape(-1), np.ones(n_steps * BL, f)])
        h0s = h0[b0 : b0 + BL]
        ht0 = np.concatenate(
            [h0s[:, 128 * k : 128 * (k + 1)].T for k in HT_ORDER], axis=1
        )
        c0s = c0[b0 : b0 + BL]
        c0l = np.vstack([c0s[:, 256 * q : 256 * (q + 1)] for q in range(4)])

        pkb = np.zeros((128, PKB_COLS), NPBF16)
        pkb[:, PKB_WT : PKB_WT + KT * 4096] = wt.astype(NPBF16)
        pkb[0:2, PKB_XB : PKB_XB + 4096] = xb.astype(NPBF16)
        pkb[:, PKB_HT0 : PKB_HT0 + KT * BL] = ht0.astype(NPBF16)
        pkb[:, PKB_IDN : PKB_IDN + 128] = idn128.astype(NPBF16)

        pkf = np.zeros((128, PKF_COLS), f)
        pkf[:, PKF_C0 : PKF_C0 + 256] = c0l
        pkf[:, PKF_WFC : PKF_WFC + 256] = wfc
        pkf[0:BL, PKF_BFC] = bfc

        in_maps.append(
            {
                "initb": np.ascontiguousarray(pkb),
                "initf": np.ascontiguousarray(pkf),
                "x1": np.ascontiguousarray(x1.astype(NPBF16)),
            }
        )
    return in_maps


def run(inputs: dict, n_steps: int = T, trace: bool = False):
    nc = build_nc(n_steps)
    in_maps = prep_inputs(**inputs, n_steps=n_steps)
    res = run_bass_kernel_spmd(nc, in_maps, list(range(NCORES)), trace=trace)
    out = np.concatenate([res.results[i]["out"] for i in range(NCORES)], axis=0)
    return out, res


def kernel(**inputs) -> np.ndarray:
    out, _ = run(inputs, n_steps=T)
    return out
